# revision 10
# baseline (speedup 1.0000x reference)
"""Trainium2 Bass kernel for a causal single-head attention block.

Problem: y = softmax(mask(Q K^T / sqrt(H))) V with
  x  [B=4, T=4096, C=1024] f32,  Wq/Wk/Wv [C, H=64] f32.

Sharding (8 NeuronCores): data-parallel over B across core pairs;
within a pair, T is split by interleaved 512-row q-tiles (rank r owns
global tiles {2s+r}) so causal work is balanced.  There is NO
collective: each core projects K^T and V for the FULL 4096 rows of its
batch (the redundant K/V compute is far cheaper than a collective in
this environment) and Q for its own 2048 rows.  K and Q are fused into
one [C,128] weight so the PE runs full width.

The compiled graph is identical on all 8 cores (SPMD).  The host
permutes x columns to [own tiles | partner tiles] so all addressing is
rank-independent; causality is delivered via data:
 - `tri`  [128,512]: lower-triangle sheet for this rank's diagonal
   chunks (identical content on both ranks, local-index addressing).
 - `kill` [128,512]: zeros on the even rank / ones on the odd rank —
   kills the causal-overhang items the uniform graph forces.

Attention items (s,i,c) for i<=s, over own chunks (phase 1) then
partner chunks (phase 2); i==s items are the diagonal (tri,
width-narrowed to 512-128c) in phase 1 and overhang (kill) in phase 2.
Exp runs on the scalar engine (groups packed to <=1024 cols, never
straddling a PSUM bank), masks on the vector engine, PV matmuls
accumulate into per-slot PSUM banks with a ones-column appended to V
so row-sums land in y_acc row 64; the division happens on the host.
PV emission lags S emission by one group so the PE never waits on the
activation.
"""

import numpy as np
import ml_dtypes

import concourse.bass as bass
import concourse.bacc as bacc
import concourse.mybir as mybir
from concourse.tile import TileContext
from concourse.bass_utils import run_bass_kernel_spmd

BF16 = mybir.dt.bfloat16
F32 = mybir.dt.float32
bf16 = ml_dtypes.bfloat16

B, T, C, H = 4, 4096, 1024, 64
N_CORES = 8
TOWN = 2048          # q rows owned per core
NSLOT = 4
QT = 512
KC = 128


def build_items():
    """Uniform (rank-independent) attention item lists.

    item = (s, i, c, width, qoff, mask) with mask in {None,'tri','kill'}.
    phase 1 = own chunks (i,c), phase 2 = partner chunks.
    """
    ph1, ph2 = [], []
    for i in range(NSLOT):
        for c in range(4):
            for s in (3, 2, 1):
                if s > i:
                    ph1.append((s, i, c, QT, 0, None))
            ph1.append((i, i, c, QT - KC * c, KC * c, "tri"))
            for s in (3, 2, 1):
                if s > i:
                    ph2.append((s, i, c, QT, 0, None))
            ph2.append((i, i, c, QT, 0, "kill"))
    return ph1, ph2


def pack_groups(items):
    # no matmul dst may straddle a 512-col (2KB) PSUM bank boundary
    groups, cur, w = [], [], 0
    for it in items:
        if w + it[3] > 1024 or (w % 512) + it[3] > 512:
            groups.append(cur)
            cur, w = [], 0
        cur.append(it)
        w += it[3]
    if cur:
        groups.append(cur)
    return groups


def build_bass():
    nc = bacc.Bacc(
        "TRN2",
        target_bir_lowering=False,
        debug=False,
        enable_asserts=False,
        num_devices=N_CORES,
    )

    xT = nc.declare_dram_parameter("xT", [C, T], BF16, isOutput=False)
    wkq = nc.declare_dram_parameter("wkq", [C, 128], BF16, isOutput=False)
    wv = nc.declare_dram_parameter("wv", [C, H], BF16, isOutput=False)
    ident = nc.declare_dram_parameter("ident", [H, H], BF16, isOutput=False)
    tri = nc.declare_dram_parameter("tri", [128, QT], BF16, isOutput=False)
    kill = nc.declare_dram_parameter("kill", [128, QT], BF16, isOutput=False)
    out = nc.declare_dram_parameter("out", [H + 1, TOWN], F32, isOutput=True)

    ph1, ph2 = build_items()
    n_items = [8 * (s + 1) for s in range(NSLOT)]

    with TileContext(nc) as tc:
        with (
            tc.tile_pool(name="persist", bufs=1) as pp,
            tc.tile_pool(name="work", bufs=3) as wp,
        ):
            # ---- persistent SBUF ----
            xT_sb = pp.tile([128, 8, T], BF16, tag="xT")
            wkq_sb = pp.tile([128, 8, 128], BF16, tag="wkq")
            wv_sb = pp.tile([128, 8, H], BF16, tag="wv")
            id_sb = pp.tile([H, H], BF16, tag="ident")
            tri_sb = pp.tile([128, QT], BF16, tag="tri")
            kill_sb = pp.tile([128, QT], BF16, tag="kill")
            qT = pp.tile([H, TOWN], BF16, tag="qT")
            kT = pp.tile([H, T], BF16, tag="kT")
            vT_all = pp.tile([H, T], BF16, tag="vTall")
            # V chunks [own 0:16 | partner 16:32], col 64 = ones
            vaug = pp.tile([128, 32, H + 1], BF16, tag="vaug")

            # ---- loads: weights first (sync), x tiles split over 2 queues
            nc.sync.dma_start(
                out=wkq_sb[:], in_=wkq[:].rearrange("(cc p) h -> p cc h", p=128)
            )
            nc.sync.dma_start(out=id_sb[:], in_=ident[:])
            nc.sync.dma_start(out=tri_sb[:], in_=tri[:])
            nc.sync.dma_start(out=kill_sb[:], in_=kill[:])
            nc.sync.dma_start(
                out=wv_sb[:], in_=wv[:].rearrange("(cc p) h -> p cc h", p=128)
            )
            for j in range(8):
                sl = slice(j * QT, (j + 1) * QT)
                eng = nc.sync if j % 2 == 0 else nc.gpsimd
                eng.dma_start(
                    out=xT_sb[:, :, sl],
                    in_=xT[:, sl].rearrange("(cc p) t -> p cc t", p=128),
                )
            nc.vector.memset(vaug[:, :, H : H + 1], 1.0)

            # preload the exp activation table while DMAs run
            dummy = wp.tile([H, H], BF16, tag="dummy")
            nc.scalar.activation(
                dummy[:], id_sb[:], mybir.ActivationFunctionType.Exp
            )

            # ---- projections (+ PE warm-up on weights during x load) ----
            with tc.tile_pool(name="proj_ps", bufs=2, space="PSUM") as pps:
                warm = pps.tile([128, QT], F32, tag="pkq", name="warm")
                for _ in range(20):
                    nc.tensor.matmul(
                        warm[:],
                        wkq_sb[:, 0, :],
                        wkq_sb[:, 0:4, :],
                        start=True,
                        stop=True,
                    )

                for j in range(8):
                    sl = slice(j * QT, (j + 1) * QT)
                    ps_kq = pps.tile([128, QT], F32, tag="pkq")
                    for cc in range(8):
                        nc.tensor.matmul(
                            ps_kq[:],
                            wkq_sb[:, cc, :],
                            xT_sb[:, cc, sl],
                            start=(cc == 0),
                            stop=(cc == 7),
                        )
                    nc.scalar.copy(kT[:, sl], ps_kq[0:H, :])
                    if j < 4:
                        nc.scalar.copy(qT[:, sl], ps_kq[H:128, :])
                    ps_v = pps.tile([H, QT], F32, tag="pv")
                    for cc in range(8):
                        nc.tensor.matmul(
                            ps_v[:],
                            wv_sb[:, cc, :],
                            xT_sb[:, cc, sl],
                            start=(cc == 0),
                            stop=(cc == 7),
                        )
                    nc.vector.tensor_copy(vT_all[:, sl], ps_v[:])
                    for c in range(4):
                        pt_t = pps.tile([128, H], BF16, tag="vt")
                        nc.tensor.transpose(
                            pt_t[:],
                            vT_all[:, j * QT + c * KC : j * QT + (c + 1) * KC],
                            id_sb[:],
                        )
                        nc.vector.tensor_copy(vaug[:, 4 * j + c, 0:H], pt_t[:])

            # ---- attention ----
            with (
                tc.tile_pool(name="swide", bufs=2, space="PSUM") as sp,
                tc.tile_pool(name="yacc", bufs=1, space="PSUM") as yp,
            ):
                y_acc = [
                    yp.tile([128, QT], F32, tag=f"y{s}", name=f"y_acc{s}")
                    for s in range(NSLOT)
                ]
                cnt = [0] * NSLOT

                def emit_pv(pend):
                    pt, metas = pend
                    for (s, i, c, w, qoff, mask), off, vbase in metas:
                        cnt[s] += 1
                        nc.tensor.matmul(
                            y_acc[s][0 : H + 1, qoff : qoff + w],
                            vaug[:, vbase + 4 * i + c, :],
                            pt[:, off : off + w],
                            start=(cnt[s] == 1),
                            stop=(cnt[s] == n_items[s]),
                        )
                        if cnt[s] == n_items[s]:
                            ysb = wp.tile([H + 1, QT], F32, tag="ysb")
                            nc.vector.tensor_copy(ysb[:], y_acc[s][0 : H + 1, :])
                            nc.sync.dma_start(
                                out=out[:, s * QT : (s + 1) * QT], in_=ysb[:]
                            )

                pend = None
                for phase, groups in (
                    (1, pack_groups(ph1)),
                    (2, pack_groups(ph2)),
                ):
                    kbase = 0 if phase == 1 else TOWN
                    vbase = 0 if phase == 1 else 16
                    for g in groups:
                        sw = sp.tile([128, 1024], F32, tag="swide")
                        metas, cur = [], 0
                        for it in g:
                            s, i, c, w, qoff, mask = it
                            kcol = kbase + KC * (4 * i + c)
                            nc.tensor.matmul(
                                sw[:, cur : cur + w],
                                kT[:, kcol : kcol + KC],
                                qT[:, s * QT + qoff : s * QT + qoff + w],
                                start=True,
                                stop=True,
                            )
                            metas.append((it, cur, vbase))
                            cur += w
                        if pend is not None:
                            emit_pv(pend)
                        pt = wp.tile([128, 1024], BF16, tag="pt")
                        nc.scalar.activation(
                            pt[:, 0:cur],
                            sw[:, 0:cur],
                            mybir.ActivationFunctionType.Exp,
                            scale=float(H) ** -0.5,
                        )
                        for (s, i, c, w, qoff, mask), off, _ in metas:
                            if mask == "tri":
                                nc.vector.tensor_mul(
                                    pt[:, off : off + w],
                                    pt[:, off : off + w],
                                    tri_sb[:, 0:w],
                                )
                            elif mask == "kill":
                                nc.vector.tensor_mul(
                                    pt[:, off : off + w],
                                    pt[:, off : off + w],
                                    kill_sb[:, 0:w],
                                )
                        pend = (pt, metas)
                emit_pv(pend)

    nc.compile()
    return nc


_NC_CACHE = None


def _get_nc():
    global _NC_CACHE
    if _NC_CACHE is None:
        _NC_CACHE = build_bass()
    return _NC_CACHE


def _make_in_maps(x, Wq, Wk, Wv):
    ident = np.eye(H, dtype=bf16)
    wkq = np.concatenate([Wk, Wq], axis=1).astype(bf16)
    wv16 = Wv.astype(bf16)
    p = np.arange(128)[:, None]
    xx = np.arange(QT)[None, :]
    tri = (p <= xx).astype(bf16)
    kills = [np.zeros((128, QT), dtype=bf16), np.ones((128, QT), dtype=bf16)]
    in_maps = []
    for c in range(N_CORES):
        b, r = divmod(c, 2)
        xbT = x[b].T.astype(bf16)  # [C, T]
        perm = np.concatenate(
            [np.arange((2 * s + rr) * QT, (2 * s + rr + 1) * QT)
             for rr in (r, 1 - r) for s in range(NSLOT)]
        )
        xT_c = np.ascontiguousarray(xbT[:, perm])
        in_maps.append(
            {
                "xT": xT_c,
                "wkq": wkq,
                "wv": wv16,
                "ident": ident,
                "tri": tri,
                "kill": kills[r],
            }
        )
    return in_maps


def _assemble(results):
    y = np.empty((B, T, H), dtype=np.float32)
    for c in range(N_CORES):
        b, r = divmod(c, 2)
        o = np.asarray(results[c]["out"], dtype=np.float32)  # [65, 2048]
        yt = o[0:H] / o[H : H + 1]
        for s in range(NSLOT):
            g = 2 * s + r
            y[b, g * QT : (g + 1) * QT] = yt[:, s * QT : (s + 1) * QT].T
    return y


def run(x, Wq, Wk, Wv, trace=False):
    nc = _get_nc()
    in_maps = _make_in_maps(
        np.asarray(x, np.float32),
        np.asarray(Wq, np.float32),
        np.asarray(Wk, np.float32),
        np.asarray(Wv, np.float32),
    )
    res = run_bass_kernel_spmd(nc, in_maps, core_ids=list(range(N_CORES)), trace=trace)
    return _assemble(res.results), res


def kernel(x, Wq, Wk, Wv):
    y, _ = run(x, Wq, Wk, Wv)
    return y


# revision 11
# speedup vs baseline: 1.0783x; 1.0783x over previous
"""Trainium2 Bass kernel for a causal single-head attention block.

Problem: y = softmax(mask(Q K^T / sqrt(H))) V with
  x  [B=4, T=4096, C=1024] f32,  Wq/Wk/Wv [C, H=64] f32.

Sharding (8 NeuronCores): data-parallel over B across core pairs;
within a pair, T is split by interleaved 512-row q-tiles (rank r owns
global tiles {2s+r}) so causal work is balanced.  There is NO
collective: each core projects K^T and V for the FULL 4096 rows of its
batch (redundant K/V compute is far cheaper than a collective here)
and Q for its own 2048 rows.  K and Q are fused into one [C,128]
weight so the PE runs full width.

The whole kernel is ONE gapless tensor-engine stream so the PE's HAM
clock gate stays un-throttled (2.4 GHz): projection of the partner
tiles is interleaved between attention groups on the own tiles, and
attention runs in two slot-subpasses ({2,3} then {0,1}) so PSUM fits:
proj(2) + transposes(2) + S(2) + y_acc(2) = 8 banks.  PV emission lags
S emission by one group so the PE never waits on the exp activation.

The compiled graph is identical on all 8 cores (SPMD).  The host
permutes x columns to [own tiles | partner tiles] so all addressing is
rank-independent; causality is delivered via data (`tri` diagonal
sheet — identical on both ranks thanks to local-index addressing — and
`kill`, zeros on the even rank, which cancels the causal-overhang
items the uniform graph forces).  Exp runs on the scalar engine, masks
on the vector engine, PV matmuls accumulate into per-slot PSUM banks
with a ones-column appended to V so row-sums land in y_acc row 64; the
division happens on the host.
"""

import numpy as np
import ml_dtypes

import concourse.bass as bass
import concourse.bacc as bacc
import concourse.mybir as mybir
from concourse.tile import TileContext
from concourse.bass_utils import run_bass_kernel_spmd

BF16 = mybir.dt.bfloat16
F32 = mybir.dt.float32
bf16 = ml_dtypes.bfloat16

B, T, C, H = 4, 4096, 1024, 64
N_CORES = 8
TOWN = 2048
NSLOT = 4
QT = 512
KC = 128

SUBPASSES = ((3, 2), (1, 0))


def subpass_items(slots, phase):
    """Uniform attention items for a slot subpass.

    item = (s, i, c, width, qoff, mask); (i,c) indexes own chunks in
    phase 'own' and partner chunks in phase 'partner'.
    """
    items = []
    for i in range(NSLOT):
        for c in range(4):
            for s in sorted((s for s in slots if s > i), reverse=True):
                items.append((s, i, c, QT, 0, None))
            if i in slots:
                if phase == "own":
                    items.append((i, i, c, QT - KC * c, KC * c, "tri"))
                else:
                    items.append((i, i, c, QT, 0, "kill"))
    return items


def pack_groups(items, lim=QT):
    # no matmul dst may straddle a 512-col (2KB) PSUM bank boundary
    groups, cur, w = [], [], 0
    for it in items:
        if w + it[3] > lim:
            groups.append(cur)
            cur, w = [], 0
        cur.append(it)
        w += it[3]
    if cur:
        groups.append(cur)
    return groups


def build_bass():
    nc = bacc.Bacc(
        "TRN2",
        target_bir_lowering=False,
        debug=False,
        enable_asserts=False,
        num_devices=N_CORES,
    )

    xT = nc.declare_dram_parameter("xT", [C, T], BF16, isOutput=False)
    wkq = nc.declare_dram_parameter("wkq", [C, 128], BF16, isOutput=False)
    wv = nc.declare_dram_parameter("wv", [C, H], BF16, isOutput=False)
    ident = nc.declare_dram_parameter("ident", [H, H], BF16, isOutput=False)
    tri = nc.declare_dram_parameter("tri", [128, QT], BF16, isOutput=False)
    kill = nc.declare_dram_parameter("kill", [128, QT], BF16, isOutput=False)
    out = nc.declare_dram_parameter("out", [H + 1, TOWN], F32, isOutput=True)

    n_items = [8 * (s + 1) for s in range(NSLOT)]

    with TileContext(nc) as tc:
        with (
            tc.tile_pool(name="persist", bufs=1) as pp,
            tc.tile_pool(name="work", bufs=3) as wp,
            tc.tile_pool(name="proj_ps", bufs=2, space="PSUM") as pps,
            tc.tile_pool(name="swide", bufs=2, space="PSUM") as sp,
            tc.tile_pool(name="yacc", bufs=1, space="PSUM") as yp,
        ):
            # ---- persistent SBUF ----
            xT_sb = pp.tile([128, 8, T], BF16, tag="xT")
            wkq_sb = pp.tile([128, 8, 128], BF16, tag="wkq")
            wv_sb = pp.tile([128, 8, H], BF16, tag="wv")
            id_sb = pp.tile([H, H], BF16, tag="ident")
            tri_sb = pp.tile([128, QT], BF16, tag="tri")
            kill_sb = pp.tile([128, QT], BF16, tag="kill")
            qT = pp.tile([H, TOWN], BF16, tag="qT")
            kT = pp.tile([H, T], BF16, tag="kT")
            vT_all = pp.tile([H, T], BF16, tag="vTall")
            # V chunks [own 0:16 | partner 16:32], col 64 = ones
            vaug = pp.tile([128, 32, H + 1], BF16, tag="vaug")

            # ---- loads: weights on sync; x tiles split over 2 queues ----
            nc.sync.dma_start(
                out=wkq_sb[:], in_=wkq[:].rearrange("(cc p) h -> p cc h", p=128)
            )
            nc.sync.dma_start(out=id_sb[:], in_=ident[:])
            nc.sync.dma_start(out=tri_sb[:], in_=tri[:])
            nc.sync.dma_start(out=kill_sb[:], in_=kill[:])
            nc.sync.dma_start(
                out=wv_sb[:], in_=wv[:].rearrange("(cc p) h -> p cc h", p=128)
            )
            for j in (0, 1, 4, 5):
                sl = slice(j * QT, (j + 1) * QT)
                nc.gpsimd.dma_start(
                    out=xT_sb[:, :, sl],
                    in_=xT[:, sl].rearrange("(cc p) t -> p cc t", p=128),
                )
            for j in (2, 3, 6, 7):
                sl = slice(j * QT, (j + 1) * QT)
                nc.sync.dma_start(
                    out=xT_sb[:, :, sl],
                    in_=xT[:, sl].rearrange("(cc p) t -> p cc t", p=128),
                )
            nc.vector.memset(vaug[:, :, H : H + 1], 1.0)

            # preload the exp activation table while DMAs run
            dummy = wp.tile([H, H], BF16, tag="dummy")
            nc.scalar.activation(
                dummy[:], id_sb[:], mybir.ActivationFunctionType.Exp
            )

            # ---- emission helpers ----
            def proj_kq(j):
                sl = slice(j * QT, (j + 1) * QT)
                ps = pps.tile([128, QT], F32, tag="proj")
                for cc in range(8):
                    nc.tensor.matmul(
                        ps[:],
                        wkq_sb[:, cc, :],
                        xT_sb[:, cc, sl],
                        start=(cc == 0),
                        stop=(cc == 7),
                    )
                nc.scalar.copy(kT[:, sl], ps[0:H, :])
                if j < 4:
                    nc.scalar.copy(qT[:, sl], ps[H:128, :])

            def proj_v(j):
                sl = slice(j * QT, (j + 1) * QT)
                ps = pps.tile([128, QT], F32, tag="proj")
                for cc in range(8):
                    nc.tensor.matmul(
                        ps[0:H, :],
                        wv_sb[:, cc, :],
                        xT_sb[:, cc, sl],
                        start=(cc == 0),
                        stop=(cc == 7),
                    )
                nc.vector.tensor_copy(vT_all[:, sl], ps[0:H, :])
                for c in range(4):
                    pt_t = pps.tile([128, H], BF16, tag="vt")
                    nc.tensor.transpose(
                        pt_t[:],
                        vT_all[:, j * QT + c * KC : j * QT + (c + 1) * KC],
                        id_sb[:],
                    )
                    nc.vector.tensor_copy(vaug[:, 4 * j + c, 0:H], pt_t[:])

            cnt = [0] * NSLOT
            y_cur = {}
            state = {"pend": None}

            def emit_pv():
                if state["pend"] is None:
                    return
                pt, metas = state["pend"]
                state["pend"] = None
                for (s, i, c, w, qoff, mask), off, vbase in metas:
                    cnt[s] += 1
                    nc.tensor.matmul(
                        y_cur[s][0 : H + 1, qoff : qoff + w],
                        vaug[:, vbase + 4 * i + c, :],
                        pt[:, off : off + w],
                        start=(cnt[s] == 1),
                        stop=(cnt[s] == n_items[s]),
                    )
                    if cnt[s] == n_items[s]:
                        ysb = wp.tile([H + 1, QT], F32, tag="ysb")
                        nc.vector.tensor_copy(ysb[:], y_cur[s][0 : H + 1, :])
                        nc.sync.dma_start(
                            out=out[:, s * QT : (s + 1) * QT], in_=ysb[:]
                        )

            def emit_group(g, phase):
                kbase = 0 if phase == "own" else TOWN
                vbase = 0 if phase == "own" else 16
                sw = sp.tile([128, QT], F32, tag="swide")
                metas, cur = [], 0
                for it in g:
                    s, i, c, w, qoff, mask = it
                    kcol = kbase + KC * (4 * i + c)
                    nc.tensor.matmul(
                        sw[:, cur : cur + w],
                        kT[:, kcol : kcol + KC],
                        qT[:, s * QT + qoff : s * QT + qoff + w],
                        start=True,
                        stop=True,
                    )
                    metas.append((it, cur, vbase))
                    cur += w
                emit_pv()
                pt = wp.tile([128, QT], BF16, tag="pt")
                nc.scalar.activation(
                    pt[:, 0:cur],
                    sw[:, 0:cur],
                    mybir.ActivationFunctionType.Exp,
                    scale=float(H) ** -0.5,
                )
                for (s, i, c, w, qoff, mask), off, _ in metas:
                    if mask == "tri":
                        nc.vector.tensor_mul(
                            pt[:, off : off + w],
                            pt[:, off : off + w],
                            tri_sb[:, 0:w],
                        )
                    elif mask == "kill":
                        nc.vector.tensor_mul(
                            pt[:, off : off + w],
                            pt[:, off : off + w],
                            kill_sb[:, 0:w],
                        )
                state["pend"] = (pt, metas)

            # ---- PE warm-up on weights during x load ----
            warm = pps.tile([128, QT], F32, tag="proj", name="warm")
            for _ in range(20):
                nc.tensor.matmul(
                    warm[:], wkq_sb[:, 0, :], wkq_sb[:, 0:4, :],
                    start=True, stop=True,
                )

            # ---- own-tile projections ----
            for j in range(4):
                proj_kq(j)
                proj_v(j)

            # ---- subpass A: slots {2,3}; partner proj interleaved ----
            for sub, slots in enumerate(SUBPASSES):
                for s in slots:
                    y_cur[s] = yp.tile(
                        [128, QT], F32, tag=f"y{s % 2}", name=f"y{sub}{s}"
                    )
                own_groups = pack_groups(subpass_items(slots, "own"))
                if sub == 0:
                    # interleave partner projections between own groups
                    gi = iter(own_groups)
                    done = False
                    for j in range(4, 8):
                        for unit in (lambda: proj_kq(j), lambda: proj_v(j)):
                            unit()
                            for _ in range(3):
                                g = next(gi, None)
                                if g is None:
                                    done = True
                                    break
                                emit_group(g, "own")
                    if not done:
                        for g in gi:
                            emit_group(g, "own")
                else:
                    for g in own_groups:
                        emit_group(g, "own")
                for g in pack_groups(subpass_items(slots, "partner")):
                    emit_group(g, "partner")
            emit_pv()

    nc.compile()
    return nc


_NC_CACHE = None


def _get_nc():
    global _NC_CACHE
    if _NC_CACHE is None:
        _NC_CACHE = build_bass()
    return _NC_CACHE


def _make_in_maps(x, Wq, Wk, Wv):
    ident = np.eye(H, dtype=bf16)
    wkq = np.concatenate([Wk, Wq], axis=1).astype(bf16)
    wv16 = Wv.astype(bf16)
    p = np.arange(128)[:, None]
    xx = np.arange(QT)[None, :]
    tri = (p <= xx).astype(bf16)
    kills = [np.zeros((128, QT), dtype=bf16), np.ones((128, QT), dtype=bf16)]
    in_maps = []
    for c in range(N_CORES):
        b, r = divmod(c, 2)
        xbT = x[b].T.astype(bf16)  # [C, T]
        perm = np.concatenate(
            [np.arange((2 * s + rr) * QT, (2 * s + rr + 1) * QT)
             for rr in (r, 1 - r) for s in range(NSLOT)]
        )
        xT_c = np.ascontiguousarray(xbT[:, perm])
        in_maps.append(
            {
                "xT": xT_c,
                "wkq": wkq,
                "wv": wv16,
                "ident": ident,
                "tri": tri,
                "kill": kills[r],
            }
        )
    return in_maps


def _assemble(results):
    y = np.empty((B, T, H), dtype=np.float32)
    for c in range(N_CORES):
        b, r = divmod(c, 2)
        o = np.asarray(results[c]["out"], dtype=np.float32)  # [65, 2048]
        yt = o[0:H] / o[H : H + 1]
        for s in range(NSLOT):
            g = 2 * s + r
            y[b, g * QT : (g + 1) * QT] = yt[:, s * QT : (s + 1) * QT].T
    return y


def run(x, Wq, Wk, Wv, trace=False):
    nc = _get_nc()
    in_maps = _make_in_maps(
        np.asarray(x, np.float32),
        np.asarray(Wq, np.float32),
        np.asarray(Wk, np.float32),
        np.asarray(Wv, np.float32),
    )
    res = run_bass_kernel_spmd(nc, in_maps, core_ids=list(range(N_CORES)), trace=trace)
    return _assemble(res.results), res


def kernel(x, Wq, Wk, Wv):
    y, _ = run(x, Wq, Wk, Wv)
    return y


# revision 17
# speedup vs baseline: 1.3368x; 1.2397x over previous
"""Trainium2 Bass kernel for a causal single-head attention block.

Problem: y = softmax(mask(Q K^T / sqrt(H))) V with
  x  [B=4, T=4096, C=1024] f32,  Wq/Wk/Wv [C, H=64] f32.

Sharding (8 NeuronCores): data-parallel over B across core pairs;
within a pair, T is split by interleaved 512-row q-tiles (rank r owns
global tiles {2s+r}) so causal work is balanced.  There is NO
collective: each core projects K^T and V for the FULL 4096 rows of its
batch (redundant K/V compute is far cheaper than a collective here)
and Q for its own 2048 rows.  K and Q are fused into one [C,128]
weight so the PE runs full width.

The whole kernel is ONE gapless tensor-engine stream so the PE's HAM
clock gate stays un-throttled (2.4 GHz): projection of the partner
tiles is interleaved between attention groups on the own tiles, and
attention runs in two slot-subpasses ({2,3} then {0,1}) so PSUM fits:
proj(2) + transposes(2) + S(2) + y_acc(2) = 8 banks.  PV emission lags
S emission by one group so the PE never waits on the exp activation.

The compiled graph is identical on all 8 cores (SPMD).  The host
permutes x columns to [own tiles | partner tiles] so all addressing is
rank-independent; causality is delivered via data (`tri` diagonal
sheet — identical on both ranks thanks to local-index addressing — and
`kill`, zeros on the even rank, which cancels the causal-overhang
items the uniform graph forces).  Exp runs on the scalar engine, masks
on the vector engine, PV matmuls accumulate into per-slot PSUM banks
with a ones-column appended to V so row-sums land in y_acc row 64; the
division happens on the host.
"""

import numpy as np
import ml_dtypes

import concourse.bass as bass
import concourse.bacc as bacc
import concourse.mybir as mybir
from concourse.tile import TileContext
from concourse.bass_utils import run_bass_kernel_spmd

BF16 = mybir.dt.bfloat16
F32 = mybir.dt.float32
bf16 = ml_dtypes.bfloat16

B, T, C, H = 4, 4096, 1024, 64
N_CORES = 8
TOWN = 2048
NSLOT = 4
QT = 512
KC = 128

SUBPASSES = ((3, 2), (1, 0))


def subpass_items(slots, phase):
    """Uniform attention items for a slot subpass.

    item = (s, i, c, width, qoff, mask); (i,c) indexes own chunks in
    phase 'own' and partner chunks in phase 'partner'.
    """
    items = []
    for i in range(NSLOT):
        for c in range(4):
            for s in sorted((s for s in slots if s > i), reverse=True):
                items.append((s, i, c, QT, 0, None))
            if i in slots:
                if phase == "own":
                    items.append((i, i, c, QT - KC * c, KC * c, "tri"))
                else:
                    items.append((i, i, c, QT, 0, "kill"))
    return items


def pack_groups(items, lim=QT):
    # no matmul dst may straddle a 512-col (2KB) PSUM bank boundary
    groups, cur, w = [], [], 0
    for it in items:
        if w + it[3] > lim:
            groups.append(cur)
            cur, w = [], 0
        cur.append(it)
        w += it[3]
    if cur:
        groups.append(cur)
    return groups


def build_bass():
    nc = bacc.Bacc(
        "TRN2",
        target_bir_lowering=False,
        debug=False,
        enable_asserts=False,
        num_devices=N_CORES,
    )

    xT = nc.declare_dram_parameter("xT", [C, T], BF16, isOutput=False)
    wkq = nc.declare_dram_parameter("wkq", [C, 128], BF16, isOutput=False)
    wv = nc.declare_dram_parameter("wv", [C, H], BF16, isOutput=False)
    ident = nc.declare_dram_parameter("ident", [H, H], BF16, isOutput=False)
    tri = nc.declare_dram_parameter("tri", [128, QT], BF16, isOutput=False)
    kill = nc.declare_dram_parameter("kill", [128, QT], BF16, isOutput=False)
    out = nc.declare_dram_parameter("out", [H + 1, TOWN], F32, isOutput=True)

    n_items = [8 * (s + 1) for s in range(NSLOT)]

    with TileContext(nc) as tc:
        with (
            tc.tile_pool(name="persist", bufs=1) as pp,
            tc.tile_pool(name="work", bufs=3) as wp,
            tc.tile_pool(name="proj_ps", bufs=2, space="PSUM") as pps,
            tc.tile_pool(name="vt_ps", bufs=1, space="PSUM") as vtp,
            tc.tile_pool(name="swide", bufs=3, space="PSUM") as sp,
            tc.tile_pool(name="yacc", bufs=1, space="PSUM") as yp,
        ):
            # ---- persistent SBUF ----
            xT_sb = pp.tile([128, 8, T], BF16, tag="xT")
            wkq_sb = pp.tile([128, 8, 128], BF16, tag="wkq")
            wv_sb = pp.tile([128, 8, H], BF16, tag="wv")
            id_sb = pp.tile([H, H], BF16, tag="ident")
            tri_sb = pp.tile([128, QT], BF16, tag="tri")
            kill_sb = pp.tile([128, QT], BF16, tag="kill")
            qT = pp.tile([H, TOWN], BF16, tag="qT")
            kT = pp.tile([H, T], BF16, tag="kT")
            vT_all = pp.tile([H, T], BF16, tag="vTall")
            # V chunks [own 0:16 | partner 16:32], col 64 = ones
            vaug = pp.tile([128, 32, H + 1], BF16, tag="vaug")

            # ---- loads: weights on sync; x tiles split over 2 queues ----
            nc.sync.dma_start(
                out=wkq_sb[:], in_=wkq[:].rearrange("(cc p) h -> p cc h", p=128)
            )
            nc.sync.dma_start(out=id_sb[:], in_=ident[:])
            nc.sync.dma_start(out=tri_sb[:], in_=tri[:])
            nc.sync.dma_start(out=kill_sb[:], in_=kill[:])
            nc.sync.dma_start(
                out=wv_sb[:], in_=wv[:].rearrange("(cc p) h -> p cc h", p=128)
            )
            for j in (0, 1, 4, 5):
                sl = slice(j * QT, (j + 1) * QT)
                nc.gpsimd.dma_start(
                    out=xT_sb[:, :, sl],
                    in_=xT[:, sl].rearrange("(cc p) t -> p cc t", p=128),
                )
            for j in (2, 3, 6, 7):
                sl = slice(j * QT, (j + 1) * QT)
                nc.sync.dma_start(
                    out=xT_sb[:, :, sl],
                    in_=xT[:, sl].rearrange("(cc p) t -> p cc t", p=128),
                )
            nc.vector.memset(vaug[:, :, H : H + 1], 1.0)

            # preload the exp activation table while DMAs run
            dummy = wp.tile([H, H], BF16, tag="dummy")
            nc.scalar.activation(
                dummy[:], id_sb[:], mybir.ActivationFunctionType.Exp
            )

            # ---- emission helpers ----
            def proj_kq(j):
                sl = slice(j * QT, (j + 1) * QT)
                ps = pps.tile([128, QT], F32, tag="proj")
                for cc in range(8):
                    nc.tensor.matmul(
                        ps[:],
                        wkq_sb[:, cc, :],
                        xT_sb[:, cc, sl],
                        start=(cc == 0),
                        stop=(cc == 7),
                    )
                nc.scalar.copy(kT[:, sl], ps[0:H, :])
                if j < 4:
                    nc.scalar.copy(qT[:, sl], ps[H:128, :])

            def proj_v(j):
                sl = slice(j * QT, (j + 1) * QT)
                ps = pps.tile([128, QT], F32, tag="proj")
                for cc in range(8):
                    nc.tensor.matmul(
                        ps[0:H, :],
                        wv_sb[:, cc, :],
                        xT_sb[:, cc, sl],
                        start=(cc == 0),
                        stop=(cc == 7),
                    )
                nc.vector.tensor_copy(vT_all[:, sl], ps[0:H, :])
                for c in range(4):
                    pt_t = vtp.tile([128, H], BF16, tag="vt")
                    nc.tensor.transpose(
                        pt_t[:],
                        vT_all[:, j * QT + c * KC : j * QT + (c + 1) * KC],
                        id_sb[:],
                    )
                    nc.vector.tensor_copy(vaug[:, 4 * j + c, 0:H], pt_t[:])

            cnt = [0] * NSLOT
            y_cur = {}
            pendq = []

            def emit_pv(flush=False):
                while len(pendq) > (0 if flush else 2):
                    pt, metas = pendq.pop(0)
                    _emit_pv1(pt, metas)

            def _emit_pv1(pt, metas):
                for (s, i, c, w, qoff, mask), off, vbase in metas:
                    cnt[s] += 1
                    nc.tensor.matmul(
                        y_cur[s][0 : H + 1, qoff : qoff + w],
                        vaug[:, vbase + 4 * i + c, :],
                        pt[:, off : off + w],
                        start=(cnt[s] == 1),
                        stop=(cnt[s] == n_items[s]),
                    )
                    if cnt[s] == n_items[s]:
                        ysb = wp.tile([H + 1, QT], F32, tag="ysb")
                        nc.vector.tensor_copy(ysb[:], y_cur[s][0 : H + 1, :])
                        nc.sync.dma_start(
                            out=out[:, s * QT : (s + 1) * QT], in_=ysb[:]
                        )

            def emit_group(g, phase):
                kbase = 0 if phase == "own" else TOWN
                vbase = 0 if phase == "own" else 16
                sw = sp.tile([128, QT], F32, tag="swide")
                metas, cur = [], 0
                for it in g:
                    s, i, c, w, qoff, mask = it
                    kcol = kbase + KC * (4 * i + c)
                    nc.tensor.matmul(
                        sw[:, cur : cur + w],
                        kT[:, kcol : kcol + KC],
                        qT[:, s * QT + qoff : s * QT + qoff + w],
                        start=True,
                        stop=True,
                    )
                    metas.append((it, cur, vbase))
                    cur += w
                pt = wp.tile([128, QT], BF16, tag="pt")
                nc.scalar.activation(
                    pt[:, 0:cur],
                    sw[:, 0:cur],
                    mybir.ActivationFunctionType.Exp,
                    scale=float(H) ** -0.5,
                )
                for (s, i, c, w, qoff, mask), off, _ in metas:
                    if mask == "tri":
                        nc.vector.tensor_mul(
                            pt[:, off : off + w],
                            pt[:, off : off + w],
                            tri_sb[:, 0:w],
                        )
                    elif mask == "kill":
                        nc.gpsimd.tensor_mul(
                            pt[:, off : off + w],
                            pt[:, off : off + w],
                            kill_sb[:, 0:w],
                        )
                pendq.append((pt, metas))
                emit_pv()

            # ---- PE warm-up on weights during x load ----
            warm = pps.tile([128, QT], F32, tag="proj", name="warm")
            for _ in range(20):
                nc.tensor.matmul(
                    warm[:], wkq_sb[:, 0, :], wkq_sb[:, 0:4, :],
                    start=True, stop=True,
                )

            # ---- own-tile projections ----
            for j in range(4):
                proj_kq(j)
                proj_v(j)

            # ---- subpass A: slots {2,3}; partner proj interleaved ----
            for sub, slots in enumerate(SUBPASSES):
                for s in slots:
                    y_cur[s] = yp.tile(
                        [128, QT], F32, tag=f"y{s % 2}", name=f"y{sub}{s}"
                    )
                own_groups = pack_groups(subpass_items(slots, "own"))
                if sub == 0:
                    # interleave partner projections between own groups
                    gi = iter(own_groups)
                    done = False
                    for j in range(4, 8):
                        for unit in (lambda: proj_kq(j), lambda: proj_v(j)):
                            unit()
                            for _ in range(3):
                                g = next(gi, None)
                                if g is None:
                                    done = True
                                    break
                                emit_group(g, "own")
                    if not done:
                        for g in gi:
                            emit_group(g, "own")
                else:
                    for g in own_groups:
                        emit_group(g, "own")
                for g in pack_groups(subpass_items(slots, "partner")):
                    emit_group(g, "partner")
            emit_pv(flush=True)

    nc.compile()
    return nc


_NC_CACHE = None


def _get_nc():
    global _NC_CACHE
    if _NC_CACHE is None:
        _NC_CACHE = build_bass()
    return _NC_CACHE


def _make_in_maps(x, Wq, Wk, Wv):
    ident = np.eye(H, dtype=bf16)
    wkq = np.concatenate([Wk, Wq], axis=1).astype(bf16)
    wv16 = Wv.astype(bf16)
    p = np.arange(128)[:, None]
    xx = np.arange(QT)[None, :]
    tri = (p <= xx).astype(bf16)
    kills = [np.zeros((128, QT), dtype=bf16), np.ones((128, QT), dtype=bf16)]
    in_maps = []
    for c in range(N_CORES):
        b, r = divmod(c, 2)
        xbT = x[b].T.astype(bf16)  # [C, T]
        perm = np.concatenate(
            [np.arange((2 * s + rr) * QT, (2 * s + rr + 1) * QT)
             for rr in (r, 1 - r) for s in range(NSLOT)]
        )
        xT_c = np.ascontiguousarray(xbT[:, perm])
        in_maps.append(
            {
                "xT": xT_c,
                "wkq": wkq,
                "wv": wv16,
                "ident": ident,
                "tri": tri,
                "kill": kills[r],
            }
        )
    return in_maps


def _assemble(results):
    y = np.empty((B, T, H), dtype=np.float32)
    for c in range(N_CORES):
        b, r = divmod(c, 2)
        o = np.asarray(results[c]["out"], dtype=np.float32)  # [65, 2048]
        yt = o[0:H] / o[H : H + 1]
        for s in range(NSLOT):
            g = 2 * s + r
            y[b, g * QT : (g + 1) * QT] = yt[:, s * QT : (s + 1) * QT].T
    return y


def run(x, Wq, Wk, Wv, trace=False):
    nc = _get_nc()
    in_maps = _make_in_maps(
        np.asarray(x, np.float32),
        np.asarray(Wq, np.float32),
        np.asarray(Wk, np.float32),
        np.asarray(Wv, np.float32),
    )
    res = run_bass_kernel_spmd(nc, in_maps, core_ids=list(range(N_CORES)), trace=trace)
    return _assemble(res.results), res


def kernel(x, Wq, Wk, Wv):
    y, _ = run(x, Wq, Wk, Wv)
    return y


# revision 25
# speedup vs baseline: 1.4946x; 1.1181x over previous
"""Trainium2 Bass kernel for a causal single-head attention block.

Problem: y = softmax(mask(Q K^T / sqrt(H))) V with
  x  [B=4, T=4096, C=1024] f32,  Wq/Wk/Wv [C, H=64] f32.

Sharding (8 NeuronCores): data-parallel over B across core pairs;
within a pair, T is split by interleaved 512-row q-tiles (rank r owns
global tiles {2s+r}) so causal work is balanced.  There is NO
collective: each core projects K^T and V for the FULL 4096 rows of its
batch (redundant K/V compute is far cheaper than a collective here)
and Q for its own 2048 rows.  K and Q are fused into one [C,128]
weight so the PE runs full width.

The whole kernel is ONE gapless tensor-engine stream so the PE's HAM
clock gate stays un-throttled (2.4 GHz): projection of the partner
tiles is interleaved between attention groups on the own tiles, and
attention runs in two slot-subpasses ({2,3} then {0,1}) so PSUM fits:
proj(2) + transposes(2) + S(2) + y_acc(2) = 8 banks.  PV emission lags
S emission by one group so the PE never waits on the exp activation.

The compiled graph is identical on all 8 cores (SPMD).  The host
permutes x columns to [own tiles | partner tiles] so all addressing is
rank-independent; causality is delivered via data (`tri` diagonal
sheet — identical on both ranks thanks to local-index addressing — and
`kill`, zeros on the even rank, which cancels the causal-overhang
items the uniform graph forces).  Exp runs on the scalar engine, masks
on the vector engine, PV matmuls accumulate into per-slot PSUM banks
with a ones-column appended to V so row-sums land in y_acc row 64; the
division happens on the host.
"""

import numpy as np
import ml_dtypes

import concourse.bass as bass
import concourse.bacc as bacc
import concourse.mybir as mybir
from concourse.tile import TileContext
from concourse.bass_utils import run_bass_kernel_spmd

BF16 = mybir.dt.bfloat16
F32 = mybir.dt.float32
bf16 = ml_dtypes.bfloat16

B, T, C, H = 4, 4096, 1024, 64
N_CORES = 8
TOWN = 2048
NSLOT = 4
QT = 512
KC = 128

SUBPASSES = ((3, 2), (1, 0))


def subpass_items(slots, phase):
    """Uniform attention items for a slot subpass.

    item = (s, i, c, width, qoff, mask); (i,c) indexes own chunks in
    phase 'own' and partner chunks in phase 'partner'.
    """
    items = []
    for i in range(NSLOT):
        for c in range(4):
            for s in sorted((s for s in slots if s > i), reverse=True):
                items.append((s, i, c, QT, 0, None))
            if i in slots:
                if phase == "own":
                    items.append((i, i, c, QT - KC * c, KC * c, "tri"))
                else:
                    items.append((i, i, c, QT, 0, "kill"))
    return items


def pack_groups(items, lim=QT):
    # no matmul dst may straddle a 512-col (2KB) PSUM bank boundary
    groups, cur, w = [], [], 0
    for it in items:
        if w + it[3] > lim:
            groups.append(cur)
            cur, w = [], 0
        cur.append(it)
        w += it[3]
    if cur:
        groups.append(cur)
    return groups


def build_bass():
    nc = bacc.Bacc(
        "TRN2",
        target_bir_lowering=False,
        debug=False,
        enable_asserts=False,
        num_devices=N_CORES,
    )

    xT = nc.declare_dram_parameter("xT", [C, T], BF16, isOutput=False)
    wkq = nc.declare_dram_parameter("wkq", [C, 128], BF16, isOutput=False)
    wv = nc.declare_dram_parameter("wv", [C, H], BF16, isOutput=False)
    ident = nc.declare_dram_parameter("ident", [H, H], BF16, isOutput=False)
    tri = nc.declare_dram_parameter("tri", [128, QT], BF16, isOutput=False)
    kill = nc.declare_dram_parameter("kill", [128, QT], BF16, isOutput=False)
    out = nc.declare_dram_parameter("out", [H + 1, TOWN], F32, isOutput=True)

    n_items = [8 * (s + 1) for s in range(NSLOT)]

    with TileContext(nc) as tc:
        with (
            tc.tile_pool(name="persist", bufs=1) as pp,
            tc.tile_pool(name="work", bufs=4) as wp,
            tc.tile_pool(name="proj_ps", bufs=2, space="PSUM") as pps,
            tc.tile_pool(name="vt_ps", bufs=1, space="PSUM") as vtp,
            tc.tile_pool(name="swide", bufs=3, space="PSUM") as sp,
            tc.tile_pool(name="yacc", bufs=1, space="PSUM") as yp,
        ):
            # ---- persistent SBUF ----
            xT_sb = pp.tile([128, 8, T], BF16, tag="xT")
            wkq_sb = pp.tile([128, 8, 128], BF16, tag="wkq")
            wv_sb = pp.tile([128, 8, H], BF16, tag="wv")
            id_sb = pp.tile([H, H], BF16, tag="ident")
            tri_sb = pp.tile([128, QT], BF16, tag="tri")
            kill_sb = pp.tile([128, QT], BF16, tag="kill")
            qT = pp.tile([H, TOWN], BF16, tag="qT")
            kT = pp.tile([H, T], BF16, tag="kT")
            vT_all = pp.tile([H, T], BF16, tag="vTall")
            # V chunks [own 0:16 | partner 16:32], col 64 = ones
            vaug = pp.tile([128, 32, H + 1], BF16, tag="vaug")

            # ---- loads: weights on sync; x tiles split over 2 queues ----
            nc.sync.dma_start(
                out=wkq_sb[:], in_=wkq[:].rearrange("(cc p) h -> p cc h", p=128)
            )
            nc.sync.dma_start(out=id_sb[:], in_=ident[:])
            nc.sync.dma_start(out=tri_sb[:], in_=tri[:])
            nc.sync.dma_start(out=kill_sb[:], in_=kill[:])
            nc.sync.dma_start(
                out=wv_sb[:], in_=wv[:].rearrange("(cc p) h -> p cc h", p=128)
            )
            for j in (0, 2, 4, 6):
                sl = slice(j * QT, (j + 1) * QT)
                nc.gpsimd.dma_start(
                    out=xT_sb[:, :, sl],
                    in_=xT[:, sl].rearrange("(cc p) t -> p cc t", p=128),
                )
            for j in (1, 3, 5, 7):
                sl = slice(j * QT, (j + 1) * QT)
                nc.sync.dma_start(
                    out=xT_sb[:, :, sl],
                    in_=xT[:, sl].rearrange("(cc p) t -> p cc t", p=128),
                )
            nc.vector.memset(vaug[:, :, H : H + 1], 1.0)

            # preload the exp activation table while DMAs run
            dummy = wp.tile([H, H], BF16, tag="dummy")
            nc.scalar.activation(
                dummy[:], id_sb[:], mybir.ActivationFunctionType.Exp
            )

            # ---- emission helpers ----
            def proj_kq(j):
                sl = slice(j * QT, (j + 1) * QT)
                ps = pps.tile([128, QT], F32, tag="proj")
                for cc in range(8):
                    nc.tensor.matmul(
                        ps[:],
                        wkq_sb[:, cc, :],
                        xT_sb[:, cc, sl],
                        start=(cc == 0),
                        stop=(cc == 7),
                    )
                nc.scalar.copy(kT[:, sl], ps[0:H, :])
                if j < 4:
                    nc.scalar.copy(qT[:, sl], ps[H:128, :])

            def proj_v(j):
                sl = slice(j * QT, (j + 1) * QT)
                ps = pps.tile([128, QT], F32, tag="proj")
                for cc in range(8):
                    nc.tensor.matmul(
                        ps[0:H, :],
                        wv_sb[:, cc, :],
                        xT_sb[:, cc, sl],
                        start=(cc == 0),
                        stop=(cc == 7),
                    )
                nc.vector.tensor_copy(vT_all[:, sl], ps[0:H, :])
                for c in range(4):
                    pt_t = vtp.tile([128, H], BF16, tag="vt")
                    nc.tensor.transpose(
                        pt_t[:],
                        vT_all[:, j * QT + c * KC : j * QT + (c + 1) * KC],
                        id_sb[:],
                    )
                    nc.vector.tensor_copy(vaug[:, 4 * j + c, 0:H], pt_t[:])

            cnt = [0] * NSLOT
            y_cur = {}
            pendq = []

            def emit_pv(flush=False):
                while len(pendq) > (0 if flush else 3):
                    pt, metas = pendq.pop(0)
                    _emit_pv1(pt, metas)

            def _emit_pv1(pt, metas):
                for (s, i, c, w, qoff, mask), off, vbase in metas:
                    cnt[s] += 1
                    nc.tensor.matmul(
                        y_cur[s][0 : H + 1, qoff : qoff + w],
                        vaug[:, vbase + 4 * i + c, :],
                        pt[:, off : off + w],
                        start=(cnt[s] == 1),
                        stop=(cnt[s] == n_items[s]),
                    )
                    if cnt[s] == n_items[s]:
                        ysb = wp.tile([H + 1, QT], F32, tag="ysb")
                        nc.vector.tensor_copy(ysb[:], y_cur[s][0 : H + 1, :])
                        nc.sync.dma_start(
                            out=out[:, s * QT : (s + 1) * QT], in_=ysb[:]
                        )

            def emit_group(g, phase):
                kbase = 0 if phase == "own" else TOWN
                vbase = 0 if phase == "own" else 16
                sw = sp.tile([128, QT], F32, tag="swide")
                metas, cur = [], 0
                for it in g:
                    s, i, c, w, qoff, mask = it
                    kcol = kbase + KC * (4 * i + c)
                    nc.tensor.matmul(
                        sw[:, cur : cur + w],
                        kT[:, kcol : kcol + KC],
                        qT[:, s * QT + qoff : s * QT + qoff + w],
                        start=True,
                        stop=True,
                    )
                    metas.append((it, cur, vbase))
                    cur += w
                pt = wp.tile([128, QT], BF16, tag="pt")
                nc.scalar.activation(
                    pt[:, 0:cur],
                    sw[:, 0:cur],
                    mybir.ActivationFunctionType.Exp,
                    scale=float(H) ** -0.5,
                )
                for (s, i, c, w, qoff, mask), off, _ in metas:
                    if mask == "tri":
                        nc.vector.tensor_mul(
                            pt[:, off : off + w],
                            pt[:, off : off + w],
                            tri_sb[:, 0:w],
                        )
                    elif mask == "kill":
                        nc.vector.tensor_mul(
                            pt[:, off : off + w],
                            pt[:, off : off + w],
                            kill_sb[:, 0:w],
                        )
                pendq.append((pt, metas))
                emit_pv()

            # ---- PE warm-up on weights during x load ----
            warm = pps.tile([128, QT], F32, tag="proj", name="warm")
            for _ in range(20):
                nc.tensor.matmul(
                    warm[:], wkq_sb[:, 0, :], wkq_sb[:, 0:4, :],
                    start=True, stop=True,
                )

            # ---- own-tile projections ----
            for j in range(4):
                proj_kq(j)
                proj_v(j)

            # ---- subpass A: slots {2,3}; partner proj interleaved ----
            for sub, slots in enumerate(SUBPASSES):
                for s in slots:
                    y_cur[s] = yp.tile(
                        [128, QT], F32, tag=f"y{s % 2}", name=f"y{sub}{s}"
                    )
                own_groups = pack_groups(subpass_items(slots, "own"))
                if sub == 0:
                    # interleave partner projections between own groups
                    gi = iter(own_groups)
                    done = False
                    for j in range(4, 8):
                        for unit in (lambda: proj_kq(j), lambda: proj_v(j)):
                            unit()
                            for _ in range(3):
                                g = next(gi, None)
                                if g is None:
                                    done = True
                                    break
                                emit_group(g, "own")
                    if not done:
                        for g in gi:
                            emit_group(g, "own")
                else:
                    for g in own_groups:
                        emit_group(g, "own")
                for g in pack_groups(subpass_items(slots, "partner")):
                    emit_group(g, "partner")
            emit_pv(flush=True)

    nc.compile()
    return nc


_NC_CACHE = None


def _get_nc():
    global _NC_CACHE
    if _NC_CACHE is None:
        _NC_CACHE = build_bass()
    return _NC_CACHE


def _make_in_maps(x, Wq, Wk, Wv):
    ident = np.eye(H, dtype=bf16)
    wkq = np.concatenate([Wk, Wq], axis=1).astype(bf16)
    wv16 = Wv.astype(bf16)
    p = np.arange(128)[:, None]
    xx = np.arange(QT)[None, :]
    tri = (p <= xx).astype(bf16)
    kills = [np.zeros((128, QT), dtype=bf16), np.ones((128, QT), dtype=bf16)]
    in_maps = []
    for c in range(N_CORES):
        b, r = divmod(c, 2)
        xbT = x[b].T.astype(bf16)  # [C, T]
        perm = np.concatenate(
            [np.arange((2 * s + rr) * QT, (2 * s + rr + 1) * QT)
             for rr in (r, 1 - r) for s in range(NSLOT)]
        )
        xT_c = np.ascontiguousarray(xbT[:, perm])
        in_maps.append(
            {
                "xT": xT_c,
                "wkq": wkq,
                "wv": wv16,
                "ident": ident,
                "tri": tri,
                "kill": kills[r],
            }
        )
    return in_maps


def _assemble(results):
    y = np.empty((B, T, H), dtype=np.float32)
    for c in range(N_CORES):
        b, r = divmod(c, 2)
        o = np.asarray(results[c]["out"], dtype=np.float32)  # [65, 2048]
        yt = o[0:H] / o[H : H + 1]
        for s in range(NSLOT):
            g = 2 * s + r
            y[b, g * QT : (g + 1) * QT] = yt[:, s * QT : (s + 1) * QT].T
    return y


def run(x, Wq, Wk, Wv, trace=False):
    nc = _get_nc()
    in_maps = _make_in_maps(
        np.asarray(x, np.float32),
        np.asarray(Wq, np.float32),
        np.asarray(Wk, np.float32),
        np.asarray(Wv, np.float32),
    )
    res = run_bass_kernel_spmd(nc, in_maps, core_ids=list(range(N_CORES)), trace=trace)
    return _assemble(res.results), res


def kernel(x, Wq, Wk, Wv):
    y, _ = run(x, Wq, Wk, Wv)
    return y
